# revision 44
# baseline (speedup 1.0000x reference)
"""Trainium2 Bass kernel for the ConvolutionalCapsule module.

Sharding: data-parallel over (batch, H-half): core k handles b = k//2,
output rows h in [6*(k%2), 6*(k%2)+6), i.e. 72 spatial positions per core.
Weights are replicated. All FLOPs run on-device; the host only does
layout/gather of inputs (patch extraction + weight transposes).

Device algorithm per core (pos = 72 positions), chunk layout keeps
(c-block-of-128) on SBUF partitions, chunks indexed t = i*3 + cb:
  iter0:  out0 = squash(mean_c preds) via one K=(c,i)=2304 matmul.
  iter1:  V[f,c,i] = sum_o W[f,c,o,i]*out0[f,o] via W-stationary matmuls
            [(4f,32)=128, c-chunk] streaming a block-diagonal out0
            (8 quartet tiles [128, 4f*72], built once, 32-aligned blocks).
          VP = V (.) P  (PSUM exit fused / split scalar+vector)
          agr[c, (f,pos)] = sum_i VP  (identity-matmul PSUM accumulation)
          e = exp(agr)  (scalar engine, fused with PSUM exit)
          Z = sum_f e ; pp = P * (1/Z)
          S_f = e_f (.) pp ; centroids via col-tiled w_r matmuls
          out = squash(centroids)
"""
import numpy as np

KH = KW = 3
B, H, WD, FIN, DIN = 4, 14, 14, 32, 8
F, C, DO, DI = 32, 288, 16, 8
NPOS = 72
CBLK = 3
NCHUNK = DI * CBLK  # 24
NQRT = 8            # f-quartets of 4
EPS = 1e-7

_CACHE: dict = {}


def _chunk_rows(t):
    i, cb = divmod(t, CBLK)
    c0 = cb * 128
    return i, c0, min(128, C - c0)


def _host_weights(Wm):
    """Wm: [F, C, DO, DI] float32 -> device weight layouts (fp16)."""
    w_r = np.zeros((NCHUNK, 128, F * DO), np.float16)
    for t in range(NCHUNK):
        i, c0, n = _chunk_rows(t)
        w_r[t, :n, :] = (
            Wm[:, c0:c0 + n, :, i].transpose(1, 0, 2).reshape(n, F * DO)
        )
    w_r = w_r.transpose(1, 0, 2).reshape(128, NCHUNK * F * DO).copy()
    # w8[(l,o-slot32), (Q,t,c)]: stationary V-matmul weights
    w8 = np.zeros((NQRT, NCHUNK, 128, 128), np.float16)
    for Q in range(NQRT):
        for t in range(NCHUNK):
            i, c0, n = _chunk_rows(t)
            for l in range(4):
                f = 4 * Q + l
                w8[Q, t, 32 * l:32 * l + DO, :n] = Wm[f, c0:c0 + n, :, i].T
    w8 = w8.transpose(2, 0, 1, 3).reshape(128, NQRT * NCHUNK * 128).copy()
    return w_r, w8


def _host_patches(x, k):
    """Patch tensor for core k in (i, c-block) chunk layout: [128, 24*72]."""
    b, hh = divmod(k, 2)
    h0 = 6 * hh
    P = np.empty((6, 12, KH, KW, FIN, DIN), np.float32)
    for kh in range(KH):
        for kw in range(KW):
            for h in range(6):
                P[h, :, kh, kw] = x[b, h0 + h + kh, kw:kw + 12]
    P = P.reshape(NPOS, C, DIN)
    p_ct = np.zeros((NCHUNK, 128, NPOS), np.float16)
    for t in range(NCHUNK):
        i, c0, n = _chunk_rows(t)
        p_ct[t, :n, :] = P[:, c0:c0 + n, i].T
    return p_ct.transpose(1, 0, 2).reshape(128, NCHUNK * NPOS).copy()


def _build():
    import concourse.bass as bass
    import concourse.bacc as bacc
    import concourse.mybir as mybir
    import concourse.tile as tile

    F16, F32 = mybir.dt.float16, mybir.dt.float32
    AX = mybir.AxisListType
    AF = mybir.ActivationFunctionType

    nc = bacc.Bacc(None, target_bir_lowering=False, debug=False)

    p_ct_d = nc.dram_tensor("p_ct", [128, NCHUNK * NPOS], F16, kind="ExternalInput")
    w_r_d = nc.dram_tensor("w_r", [128, NCHUNK * F * DO], F16, kind="ExternalInput")
    w8_d = nc.dram_tensor("w8", [128, NQRT * NCHUNK * 128], F16, kind="ExternalInput")
    eye72_d = nc.dram_tensor("eye72", [NPOS, NPOS], F32, kind="ExternalInput")
    eye128h_d = nc.dram_tensor("eye128h", [128, 128], F16, kind="ExternalInput")
    eye128f_d = nc.dram_tensor("eye128f", [128, 128], F32, kind="ExternalInput")
    y_d = nc.dram_tensor("y", [NPOS, F * DO], F32, kind="ExternalOutput")

    QBD = 4 * NPOS            # 288 = 4f * 72pos, one quartet's free extent

    with tile.TileContext(nc) as tc:
        with (
            tc.tile_pool(name="const", bufs=1) as const,
            tc.tile_pool(name="work", bufs=1) as work,
            tc.tile_pool(name="ring", bufs=2) as ring,
            tc.tile_pool(name="sring", bufs=3) as sring,
            tc.tile_pool(name="vsr", bufs=4) as vsr,
            tc.tile_pool(name="vps", bufs=2, space=bass.MemorySpace.PSUM) as vps,
            tc.tile_pool(name="tps", bufs=1, space=bass.MemorySpace.PSUM) as tps,
            tc.tile_pool(name="acc", bufs=2, space=bass.MemorySpace.PSUM) as acc,
            tc.tile_pool(name="cenp", bufs=1, space=bass.MemorySpace.PSUM) as cenp,
        ):
            # ---------------- loads (split so stage B starts early) ----------------
            p_ct = const.tile([128, NCHUNK * NPOS], F16, tag="p_ct")
            w_r = const.tile([128, NCHUNK * F * DO], F16, tag="w_r")
            PCS = NCHUNK * NPOS // 2
            WRS = NCHUNK * F * DO // 3
            nc.sync.dma_start(p_ct[:, 0:PCS], p_ct_d[:, 0:PCS])
            nc.sync.dma_start(w_r[:, 0:WRS], w_r_d[:, 0:WRS])
            nc.sync.dma_start(p_ct[:, PCS:2 * PCS], p_ct_d[:, PCS:2 * PCS])
            for s in range(1, 3):
                nc.sync.dma_start(
                    w_r[:, s * WRS:(s + 1) * WRS], w_r_d[:, s * WRS:(s + 1) * WRS]
                )
            eye72 = const.tile([NPOS, NPOS], F32, tag="eye72")
            nc.sync.dma_start(eye72[:], eye72_d[:])
            eye128h = const.tile([128, 128], F16, tag="eye128h")
            nc.sync.dma_start(eye128h[:], eye128h_d[:])
            eye128f = const.tile([128, 128], F32, tag="eye128f")
            nc.sync.dma_start(eye128f[:], eye128f_d[:])
            w8 = const.tile([128, NQRT * NCHUNK * 128], F16, tag="w8")
            W8S = NQRT * NCHUNK * 128 // 4
            for s in range(4):
                nc.sync.dma_start(
                    w8[:, s * W8S:(s + 1) * W8S], w8_d[:, s * W8S:(s + 1) * W8S]
                )

            epsb = const.tile([NPOS, 1], F32, tag="epsb")
            nc.vector.memset(epsb[:], EPS)

            def squash(src_ap, dst_ap, pre_scale, tag, nf=F):
                """dst = squash(src * pre_scale) ; src free = (nf, DO).

                factor = sn/((1+sn)*sqrt(sn+eps)), sn = |pre_scale*src|^2;
                dst = (pre_scale*src) * factor via one scalar_tensor_tensor.
                """
                ALU = mybir.AluOpType
                sq = work.tile([NPOS, nf * DO], F32, tag=f"{tag}_sq")
                nc.scalar.activation(sq[:], src_ap, AF.Square, scale=pre_scale)
                sn = work.tile([NPOS, nf], F32, tag=f"{tag}_sn")
                nc.vector.reduce_sum(
                    sn[:], sq[:].rearrange("p (f o) -> p f o", o=DO), axis=AX.X
                )
                t1 = work.tile([NPOS, nf], F32, tag=f"{tag}_t1")
                nc.vector.tensor_scalar_add(t1[:], sn[:], 1.0)
                r2 = work.tile([NPOS, nf], F32, tag=f"{tag}_r2")
                nc.scalar.activation(r2[:], sn[:], AF.Sqrt, bias=epsb[:])
                m = work.tile([NPOS, nf], F32, tag=f"{tag}_m")
                nc.vector.tensor_mul(m[:], t1[:], r2[:])
                r = work.tile([NPOS, nf], F32, tag=f"{tag}_r")
                nc.vector.reciprocal(r[:], m[:])
                sc = work.tile([NPOS, nf], F32, tag=f"{tag}_sc")
                nc.vector.tensor_mul(sc[:], sn[:], r[:])
                bc = sc[:].unsqueeze(2).broadcast_to((NPOS, nf, DO))
                nc.vector.scalar_tensor_tensor(
                    dst_ap, src_ap, float(pre_scale), bc,
                    op0=ALU.mult, op1=ALU.mult,
                )

            # ---------------- stage B: out0 ----------------
            o0p = acc.tile([NPOS, F * DO], F32, tag="mm")
            for t in range(NCHUNK):
                nc.tensor.matmul(
                    o0p[:],
                    p_ct[:, t * NPOS:(t + 1) * NPOS],
                    w_r[:, t * F * DO:(t + 1) * F * DO],
                    start=(t == 0),
                    stop=(t == NCHUNK - 1),
                )
            out0_pad = work.tile([NPOS, F * 32], F32, tag="out0_pad")
            nc.vector.memset(out0_pad[:], 0.0)
            squash(
                o0p[:],
                out0_pad[:].rearrange("p (f s) -> p f s", s=32)[:, :, 0:DO],
                1.0 / F,
                "sq1",
            )

            # transposes -> tpq [128=(4l,32slot), 8Q*72], block-diag bd
            tpq = work.tile([128, NQRT * NPOS], F16, tag="tpq")
            for Q in range(NQRT):
                tp = tps.tile([128, 128], F32, tag="tp")
                nc.tensor.transpose(
                    tp[:, 0:NPOS], out0_pad[:, Q * 128:(Q + 1) * 128], eye72[:]
                )
                nc.scalar.copy(tpq[:, Q * NPOS:(Q + 1) * NPOS], tp[:, 0:NPOS])
            bd = work.tile([128, NQRT * QBD], F16, tag="bd")
            nc.vector.memset(bd[:], 0.0)
            for Q in range(NQRT):
                for l in range(4):
                    nc.vector.tensor_copy(
                        bd[32 * l:32 * l + 32,
                           Q * QBD + l * NPOS:Q * QBD + (l + 1) * NPOS],
                        tpq[32 * l:32 * l + 32, Q * NPOS:(Q + 1) * NPOS],
                    )

            # ---------------- V + VP + agr + exp (software-pipelined) ----------------
            # e[c; (Q,cb,l,pos)] = exp(agr), SBUF fp16
            e = work.tile([128, NQRT * CBLK * QBD], F16, tag="e")

            def ired_phase(Q, VP):
                # agr = sum_i VP (identity-matmul accumulation), then exp
                for cb in range(CBLK):
                    agp = acc.tile([128, QBD], F32, tag="mm")
                    for i in range(DI):
                        t = i * CBLK + cb
                        nc.tensor.matmul(
                            agp[:],
                            eye128h[:],
                            VP[:, t * QBD:(t + 1) * QBD],
                            start=(i == 0),
                            stop=(i == DI - 1),
                        )
                    nc.scalar.activation(
                        e[:, (Q * CBLK + cb) * QBD:(Q * CBLK + cb + 1) * QBD],
                        agp[:],
                        AF.Exp,
                    )

            CN = CBLK * NPOS
            Zp = work.tile([128, NQRT * CN], F32, tag="Zp")
            Z4 = work.tile([128, 4 * CN], F32, tag="Z4")
            Z2 = work.tile([128, 2 * CN], F32, tag="Z2")

            def emit_zp(Q):
                # Zp[Q] = sum_l e[Q, :, l, :]; fold Z4/Z2 as pairs complete
                eq = e[:, Q * CBLK * QBD:(Q + 1) * CBLK * QBD].rearrange(
                    "p (cb l n) -> p cb l n", cb=CBLK, l=4
                )
                za = work.tile([128, CN], F32, tag="za")
                nc.vector.tensor_add(
                    za[:].rearrange("p (cb n) -> p cb n", cb=CBLK),
                    eq[:, :, 0, :], eq[:, :, 1, :],
                )
                zb = work.tile([128, CN], F32, tag="zb")
                nc.vector.tensor_add(
                    zb[:].rearrange("p (cb n) -> p cb n", cb=CBLK),
                    eq[:, :, 2, :], eq[:, :, 3, :],
                )
                nc.vector.tensor_add(
                    Zp[:, Q * CN:(Q + 1) * CN], za[:], zb[:]
                )
                if Q % 2 == 1:
                    mh = Q // 2
                    nc.vector.tensor_add(
                        Z4[:, mh * CN:(mh + 1) * CN],
                        Zp[:, (Q - 1) * CN:Q * CN],
                        Zp[:, Q * CN:(Q + 1) * CN],
                    )
                if Q == 3 or Q == 7:
                    mh = Q // 4
                    nc.vector.tensor_add(
                        Z2[:, mh * CN:(mh + 1) * CN],
                        Z4[:, 2 * mh * CN:(2 * mh + 1) * CN],
                        Z4[:, (2 * mh + 1) * CN:(2 * mh + 2) * CN],
                    )

            VPs = []
            agps = {}

            def tree_phase(Q):
                # i-sum via DVE pair-tree (replaces 24 identity matmuls)
                v4 = VPs[Q][:].rearrange("p (i cb x) -> p i cb x", i=DI, cb=CBLK)
                for cb in range(CBLK):
                    tr1 = work.tile([128, 4 * QBD], F16, tag="tr1")
                    nc.vector.tensor_add(
                        tr1[:].rearrange("p (i x) -> p i x", i=4),
                        v4[:, 0:4, cb, :], v4[:, 4:8, cb, :],
                    )
                    t1v = tr1[:].rearrange("p (i x) -> p i x", i=4)
                    tr2 = work.tile([128, 2 * QBD], F16, tag="tr2")
                    nc.vector.tensor_add(
                        tr2[:].rearrange("p (i x) -> p i x", i=2),
                        t1v[:, 0:2, :], t1v[:, 2:4, :],
                    )
                    agrT = work.tile([128, QBD], F16, tag="agrT")
                    nc.vector.tensor_add(
                        agrT[:], tr2[:, 0:QBD], tr2[:, QBD:2 * QBD]
                    )
                    nc.scalar.activation(
                        e[:, (Q * CBLK + cb) * QBD:(Q * CBLK + cb + 1) * QBD],
                        agrT[:],
                        AF.Exp,
                    )

            for Q in range(NQRT):
                VP = ring.tile([128, NCHUNK * QBD], F16, tag="VP")
                for m in range(NCHUNK // 2):
                    t = 2 * m
                    # pair of V matmuls into one 2-bank PSUM tile (512-strided)
                    vh = vps.tile([128, 1024], F32, tag="vh")
                    for k in range(2):
                        nc.tensor.matmul(
                            vh[:, k * 512:k * 512 + QBD],
                            w8[:, (Q * NCHUNK + t + k) * 128:
                               (Q * NCHUNK + t + k + 1) * 128],
                            bd[:, Q * QBD:(Q + 1) * QBD],
                            start=True,
                            stop=True,
                        )
                    # VP = V * P (P broadcast over the 4 f's of the quartet)
                    pbp = (
                        p_ct[:, t * NPOS:(t + 2) * NPOS]
                        .rearrange("p (k n) -> p k n", k=2)
                        .unsqueeze(2)
                        .broadcast_to((128, 2, 4, NPOS))
                    )
                    vp_dst = VP[:, t * QBD:(t + 2) * QBD].rearrange(
                        "p (k j n) -> p k j n", k=2, j=4
                    )
                    if m % 4 == 3:
                        nc.vector.tensor_mul(
                            vp_dst,
                            vh[:].rearrange("p (k s) -> p k s", k=2)[:, :, 0:QBD]
                            .rearrange("p k (j n) -> p k j n", j=4),
                            pbp,
                        )
                    else:
                        vs = vsr.tile([128, 2 * QBD], F16, tag="vs")
                        nc.scalar.copy(
                            vs[:].rearrange("p (k s) -> p k s", k=2),
                            vh[:].rearrange("p (k s) -> p k s", k=2)[:, :, 0:QBD],
                        )
                        nc.vector.tensor_mul(
                            vp_dst,
                            vs[:].rearrange("p (k j n) -> p k j n", k=2, j=4),
                            pbp,
                        )
                    if Q > 0:
                        for k2 in range(2):
                            idx = 2 * m + k2
                            cb_p, i_p = divmod(idx, DI)
                            if i_p == 0:
                                agt = acc.tile([128, QBD], F32, tag="mm")
                                agps[cb_p] = agt
                            tp_p = i_p * CBLK + cb_p
                            nc.tensor.matmul(
                                agps[cb_p][:],
                                eye128h[:],
                                VPs[Q - 1][:, tp_p * QBD:(tp_p + 1) * QBD],
                                start=(i_p == 0),
                                stop=(i_p == DI - 1),
                                skip_group_check=True,
                            )
                            if i_p == DI - 1:
                                nc.scalar.activation(
                                    e[:, ((Q - 1) * CBLK + cb_p) * QBD:
                                       ((Q - 1) * CBLK + cb_p + 1) * QBD],
                                    agps[cb_p][:],
                                    AF.Exp,
                                )
                VPs.append(VP)
                if Q >= 1:
                    emit_zp(Q - 1)
            ired_phase(NQRT - 1, VPs[NQRT - 1])
            emit_zp(NQRT - 1)

            # ---------------- softmax normalizer + pp ----------------
            Zf = work.tile([128, CN], F32, tag="Zf")
            nc.vector.tensor_add(Zf[:], Z2[:, 0:CN], Z2[:, CN:2 * CN])
            Zr = work.tile([128, CN], F16, tag="Zr")
            with nc.allow_low_precision(reason="Z in [F/e, F*e]; fp16 1/Z ok"):
                nc.vector.reciprocal(Zr[:], Zf[:])
            pp = work.tile([128, NCHUNK * NPOS], F16, tag="pp")
            nc.vector.tensor_mul(
                pp[:].rearrange("p (i cb n) -> p i cb n", i=DI, cb=CBLK),
                p_ct[:].rearrange("p (i cb n) -> p i cb n", i=DI, cb=CBLK),
                Zr[:].rearrange("p (cb n) -> p cb n", cb=CBLK)
                .unsqueeze(1)
                .broadcast_to((128, DI, CBLK, NPOS)),
            )

            # ---------------- S + centroids ----------------
            y_sb = work.tile([NPOS, F * DO], F32, tag="y_sb")
            cen_sb = work.tile([128, 8 * NPOS], F32, tag="cen_sb")
            opre = work.tile([NPOS, 8 * 128], F32, tag="opre")
            for g in range(8):
                cg = cenp.tile([128, NPOS], F32, tag="cen")
                Ss = []
                for j in range(4):
                    S = sring.tile([128, NCHUNK * NPOS], F16, tag=f"S{j}")
                    eb = (
                        e[:, g * CBLK * QBD:(g + 1) * CBLK * QBD]
                        .rearrange("p (cb l n) -> p cb l n", cb=CBLK, l=4)
                        [:, :, j, :]
                        .unsqueeze(1)
                        .broadcast_to((128, DI, CBLK, NPOS))
                    )
                    nc.vector.tensor_mul(
                        S[:].rearrange("p (i cb n) -> p i cb n", i=DI, cb=CBLK),
                        pp[:].rearrange("p (i cb n) -> p i cb n", i=DI, cb=CBLK),
                        eb,
                    )
                    Ss.append(S)
                for t in range(NCHUNK):
                    for j in range(4):
                        f = 4 * g + j
                        nc.tensor.matmul(
                            cg[32 * j:32 * j + DO, :],
                            w_r[:, t * F * DO + f * DO:t * F * DO + (f + 1) * DO],
                            Ss[j][:, t * NPOS:(t + 1) * NPOS],
                            start=(t == 0),
                            stop=(t == NCHUNK - 1),
                            tile_position=(0, 32 * j),
                        )
                nc.scalar.copy(cen_sb[:, g * NPOS:(g + 1) * NPOS], cg[:])
                tp2 = tps.tile([128, 128], F32, tag="tp")
                nc.tensor.transpose(
                    tp2[0:NPOS, :], cen_sb[:, g * NPOS:(g + 1) * NPOS], eye128f[:]
                )
                nc.scalar.copy(opre[:, g * 128:(g + 1) * 128], tp2[0:NPOS, :])
                if g == 3 or g == 7:
                    half = g // 4
                    squash(
                        opre[:, half * 512:(half + 1) * 512]
                        .rearrange("p (g j s) -> p g j s", g=4, j=4)
                        [:, :, :, 0:DO],
                        y_sb[:, half * 256:(half + 1) * 256]
                        .rearrange("p (f o) -> p f o", o=DO),
                        1.0,
                        f"sq2_{half}",
                        nf=16,
                    )
                    nc.sync.dma_start(
                        y_d[:, half * 256:(half + 1) * 256],
                        y_sb[:, half * 256:(half + 1) * 256],
                    )

    nc.compile()
    return nc


def _get_program():
    if "nc" not in _CACHE:
        _CACHE["nc"] = _build()
    return _CACHE["nc"]


def _in_maps(x, Wm):
    w_r, w8 = _host_weights(Wm)
    eye72 = np.eye(NPOS, dtype=np.float32)
    eye128h = np.eye(128, dtype=np.float16)
    eye128f = np.eye(128, dtype=np.float32)
    return [{
        "p_ct": _host_patches(x, k),
        "w_r": w_r,
        "w8": w8,
        "eye72": eye72,
        "eye128h": eye128h,
        "eye128f": eye128f,
    } for k in range(8)]


def kernel(x, W):
    from concourse.bass_utils import run_bass_kernel_spmd

    x = np.asarray(x, np.float32)
    Wm = np.asarray(W, np.float32)[0, 0, 0]
    nc = _get_program()
    res = run_bass_kernel_spmd(nc, _in_maps(x, Wm), list(range(8)))
    Ho, Wo = H - KH + 1, WD - KW + 1
    y = np.empty((B, Ho, Wo, F, DO), np.float32)
    for k in range(8):
        b, hh = divmod(k, 2)
        y[b, 6 * hh:6 * hh + 6] = res.results[k]["y"].reshape(6, Wo, F, DO)
    return y

